# revision 6
# baseline (speedup 1.0000x reference)
"""DiffJPEG (quality=75) Bass kernel for Trainium2, 8-core data-parallel.

v2 pipeline per image — zero PE transposes, both transpose stages fused
into neighboring matmuls via the stationary operand:
  conv:  x f32 -> fp16 tiles (input precision: fp16, validated rel<0.01)
  A+T1:  t1 = (rowDCT+color @ X).T computed directly as
         X_block.T @ (w*255*BD).T per output block (fp16 matmuls),
         Y level shift (-362.039) folded into the t1 evac DC columns.
  B:     col-DCT (+col-pool for chroma), f32r 512-wide matmuls.
  Q:     q1 = P*recip (DVE), q2 = +MAGIC (ACT copy w/ float bias),
         q3 = (q2-MAGIC)*q -> fp16 (DVE stt); tables are [128,8] tiles
         broadcast along the free dim via stride-0 APs.
  C+T2:  t2 = cq.T @ IDCT-consts per block (fp16), +128 output level
         folded into the Y t2-evac bias (per-partition, ACT Identity).
  D:     col-IDCT + color + upsample folds, consts pre-scaled 1/255 so
         PSUM holds final pixels in [0,1]; fp16 matmuls, 512-wide.
  fin:   single (min 1, max 0) tensor_scalar per chunk, then DMA out.
"""
import sys

sys.path.insert(0, "/opt/trn_rl_repo")

import numpy as np

QUALITY = 75
FACTOR = (200.0 - 2.0 * QUALITY) / 100.0  # 0.5
MAGIC = np.float32(1.5 * 2.0 ** 23)
LS = np.float64(128.0 * 8.0 * 0.5 / np.sqrt(2.0))  # 362.0386719675...

Y_TABLE = np.array([
    [16, 11, 10, 16, 24, 40, 51, 61],
    [12, 12, 14, 19, 26, 58, 60, 55],
    [14, 13, 16, 24, 40, 57, 69, 56],
    [14, 17, 22, 29, 51, 87, 80, 62],
    [18, 22, 37, 56, 68, 109, 103, 77],
    [24, 35, 55, 64, 81, 104, 113, 92],
    [49, 64, 78, 87, 103, 121, 120, 101],
    [72, 92, 95, 98, 112, 100, 103, 99]], dtype=np.float64)

C_TABLE = np.array([
    [17, 18, 24, 47, 99, 99, 99, 99],
    [18, 21, 26, 66, 99, 99, 99, 99],
    [24, 26, 56, 99, 99, 99, 99, 99],
    [47, 66, 99, 99, 99, 99, 99, 99],
    [99, 99, 99, 99, 99, 99, 99, 99],
    [99, 99, 99, 99, 99, 99, 99, 99],
    [99, 99, 99, 99, 99, 99, 99, 99],
    [99, 99, 99, 99, 99, 99, 99, 99]], dtype=np.float64)

W_FWD = {
    "y": (0.299, 0.587, 0.114),
    "cb": (-0.168736, -0.331264, 0.5),
    "cr": (0.5, -0.418688, -0.081312),
}
W_BWD = {
    "r": {"cr": 1.402},
    "g": {"cb": -0.344136, "cr": -0.714136},
    "b": {"cb": 1.772},
}

N_CORES = 8
IMGS_PER_CORE = 2
H = W = 512


def _round_f32r(x):
    """Round f32 to the 12-explicit-mantissa-bit f32r grid (RNE)."""
    x = np.ascontiguousarray(x, dtype=np.float32)
    u = x.view(np.uint32).astype(np.uint64)
    drop = 11
    half = np.uint64(1 << (drop - 1))
    low = u & np.uint64((1 << drop) - 1)
    u_hi = u >> np.uint64(drop)
    up = (low > half) | ((low == half) & ((u_hi & np.uint64(1)) == 1))
    u2 = (u_hi + up.astype(np.uint64)) << np.uint64(drop)
    return (u2 & np.uint64(0xFFFFFFFF)).astype(np.uint32).view(np.float32)


def _dct_mat():
    xg = np.arange(8, dtype=np.float64)
    ug = np.arange(8, dtype=np.float64)
    Dm = 0.5 * np.cos((2.0 * xg[None, :] + 1.0) * ug[:, None] * np.pi / 16.0)
    Dm[0, :] *= 1.0 / np.sqrt(2.0)
    return Dm


def _constants():
    D8 = _dct_mat()
    BD128 = np.kron(np.eye(16), D8)  # [128,128]
    P = np.zeros((128, 256))
    idx = np.arange(128)
    P[idx, 2 * idx] = 0.5
    P[idx, 2 * idx + 1] = 0.5
    M = np.kron(np.eye(16), D8) @ P  # [128, 256] row-pool + DCT
    P0, P1 = M[:, :128], M[:, 128:]

    # f32r pack: B-stage stationaries
    b_y = _round_f32r(BD128.T)
    b_c_k0 = _round_f32r(P0.T)
    b_c_k1 = _round_f32r(P1.T)
    pack_r = np.concatenate([b_y, b_c_k0, b_c_k1], axis=1)  # [128, 384]

    # fp16 pack: CT2 moving consts
    bd = np.asarray(BD128, dtype=np.float16)
    cc0 = np.asarray(2.0 * P0, dtype=np.float16)
    cc1 = np.asarray(2.0 * P1, dtype=np.float16)
    pack_h = np.concatenate([bd, cc0, cc1], axis=1)  # [128, 384] fp16

    # f32 pack: quant tables [128,8] x4 + bias_y [128,1]
    qy = np.tile((Y_TABLE.T * FACTOR), (16, 1)).astype(np.float32)
    qc = np.tile((C_TABLE.T * FACTOR), (16, 1)).astype(np.float32)
    ry = (1.0 / qy).astype(np.float32)
    rc = (1.0 / qc).astype(np.float32)
    bias_y = np.zeros((128, 1), dtype=np.float32)
    bias_y[0::8, 0] = np.float32(LS)
    pack_f = np.concatenate([qy, ry, qc, rc, bias_y], axis=1)  # [128, 33]

    return (np.ascontiguousarray(pack_r, dtype=np.float32),
            np.ascontiguousarray(pack_h, dtype=np.float16),
            np.ascontiguousarray(pack_f, dtype=np.float32))


_PACK_R, _PACK_H, _PACK_F = _constants()
_PROGRAM = None
TRACE = False
LAST_RESULT = None


def _build_program():
    import concourse.bacc as bacc
    import concourse.mybir as mybir
    from concourse.tile import TileContext

    f32 = mybir.dt.float32
    f32r = mybir.dt.float32r
    f16 = mybir.dt.float16
    ACT_COPY = mybir.ActivationFunctionType.Copy
    ACT_IDENT = mybir.ActivationFunctionType.Identity
    ADD = mybir.AluOpType.add
    SUB = mybir.AluOpType.subtract
    MULT = mybir.AluOpType.mult
    MIN = mybir.AluOpType.min
    MAX = mybir.AluOpType.max

    nc = bacc.Bacc("TRN2", target_bir_lowering=False, debug=False,
                   num_devices=N_CORES)

    x_d = nc.dram_tensor("xc", [IMGS_PER_CORE, 3, H, W], f32,
                         kind="ExternalInput").ap()
    out_d = nc.dram_tensor("outc", [IMGS_PER_CORE, 3, H, W], f32,
                           kind="ExternalOutput").ap()
    packr_d = nc.dram_tensor("pack_r", list(_PACK_R.shape), f32,
                             kind="ExternalInput").ap()
    packh_d = nc.dram_tensor("pack_h", list(_PACK_H.shape), f16,
                             kind="ExternalInput").ap()
    packf_d = nc.dram_tensor("pack_f", list(_PACK_F.shape), f32,
                             kind="ExternalInput").ap()

    with TileContext(nc) as tc:
        with (
            tc.tile_pool(name="const", bufs=1) as cpool,
            tc.tile_pool(name="data", bufs=2) as dpool,
            tc.tile_pool(name="work", bufs=2) as wpool,
            tc.tile_pool(name="psA", bufs=2, space="PSUM") as psA,
            tc.tile_pool(name="psB", bufs=2, space="PSUM") as psB,
            tc.tile_pool(name="psT", bufs=2, space="PSUM") as psT,
            tc.tile_pool(name="psD", bufs=2, space="PSUM") as psD,
        ):
            # ---- PE warmup: dummy matmuls while DMAs are in flight ----
            wu0 = cpool.tile([128, 16], f32, name="wu0")
            nc.gpsimd.memset(wu0[:], 1.0)
            wu = cpool.tile([128, 16], f32r, name="wu")
            nc.gpsimd.tensor_copy(wu[:], wu0[:])

            # ---- constant DMAs (ACT queue: keeps SP free for the input
            # stream and DVE free for the warmup chain) ----
            cr_t = cpool.tile([128, 384], f32r, name="cr_t")
            nc.scalar.dma_start(cr_t[:], packr_d.bitcast(f32r))
            ch_t = cpool.tile([128, 384], f16, name="ch_t")
            nc.scalar.dma_start(ch_t[:], packh_d)
            cf_t = cpool.tile([128, 33], f32, name="cf_t")
            nc.scalar.dma_start(cf_t[:], packf_d)

            cs = {
                "b_y": cr_t[:, 0:128],
                "b_c_k0": cr_t[:, 128:256],
                "b_c_k1": cr_t[:, 256:384],
                "bd": ch_t[:, 0:128],
                "cc0": ch_t[:, 128:256],
                "cc1": ch_t[:, 256:384],
                "qy": cf_t[:, 0:8],
                "ry": cf_t[:, 8:16],
                "qc": cf_t[:, 16:24],
                "rc": cf_t[:, 24:32],
                "bias_y": cf_t[:, 32:33],
            }

            wp = psA.tile([128, W], f32, name="wp", tag="psA")
            for _ in range(110):
                nc.tensor.matmul(wp[:16, 0:16], wu[:], wu[:], start=True,
                                 stop=True)

            # ---- on-chip generated fp16 consts ----
            def gen16(key, src_ap, factor, width=128):
                t = cpool.tile([128, width], f16, name=f"g_{key}")
                nc.vector.tensor_scalar_mul(t[:], src_ap, float(factor))
                cs[key] = t[:]

            for wname, wv in zip("rgb", W_FWD["y"]):
                gen16(f"ay_{wname}", cs["b_y"], wv * 255.0)
            for cn in ("cb", "cr"):
                for wname, wv in zip("rgb", W_FWD[cn]):
                    gen16(f"a_{cn}_{wname}_k0", cs["b_c_k0"][:, 0:64],
                          wv * 255.0, width=64)
                    gen16(f"a_{cn}_{wname}_k1", cs["b_c_k1"][:, 64:128],
                          wv * 255.0, width=64)
            gen16("dd_y", cs["bd"], 1.0 / 255.0)
            for och, terms in W_BWD.items():
                for cch, wv in terms.items():
                    for k in (0, 1):
                        gen16(f"d_{och}_{cch}_k{k}", cs[f"cc{k}"],
                              wv / 255.0)

            def bc8(key, reps):
                """broadcast a [128,8] table along new dim: [128,*reps,8]."""
                ap = cs[key]
                for _ in range(len(reps)):
                    ap = ap.unsqueeze(1)
                return ap.broadcast_to([128, *reps, 8])

            def mm(out_ps, lhsT_ap, rhs_ap, start, stop):
                nc.tensor.matmul(out_ps, lhsT_ap, rhs_ap,
                                 start=start, stop=stop)

            S = [{} for _ in range(IMGS_PER_CORE)]

            # ---------------- stages ----------------
            def st_load(img):
                xt = dpool.tile([128, 3, 4, W], f32, name=f"xt_{img}",
                                tag="xt")
                for k in range(4):
                    for ch in range(3):
                        nc.sync.dma_start(
                            xt[:, ch, k, :],
                            x_d[img, ch, 128 * k:128 * (k + 1), :])
                S[img]["xt"] = xt

            CONV_ENG = ("act", "pool", "dve", "pool", "act", "pool",
                        "dve", "pool", "act", "pool", "act", "pool")

            def st_conv(img, chans=(0, 1, 2)):
                xt = S[img]["xt"]
                x16 = S[img].get("x16")
                if x16 is None:
                    x16 = dpool.tile([128, 3, 4, W], f16, name=f"x16_{img}",
                                     tag="x16")
                    S[img]["x16"] = x16
                for k in range(4):
                    for ch in chans:
                        eng = CONV_ENG[(k * 3 + ch) % len(CONV_ENG)]
                        if eng == "act":
                            nc.scalar.activation(x16[:, ch, k, :],
                                                 xt[:, ch, k, :], ACT_COPY)
                        elif eng == "dve":
                            nc.vector.tensor_copy(x16[:, ch, k, :],
                                                  xt[:, ch, k, :])
                        else:
                            nc.gpsimd.tensor_copy(x16[:, ch, k, :],
                                                  xt[:, ch, k, :])

            def st_AT1y(img, j):
                """fused A+T1 for Y, output column-chunk j -> t1y[j]."""
                x16 = S[img]["x16"]
                pa = psA.tile([128, W], f32, name=f"AT1y_{img}_{j}",
                              tag="psA")
                for i in range(4):
                    for ci, wname in enumerate("rgb"):
                        mm(pa[:, 128 * i:128 * (i + 1)],
                           x16[:, ci, i, 128 * j:128 * (j + 1)],
                           cs[f"ay_{wname}"], ci == 0, ci == 2)
                t1 = wpool.tile([128, W], f32r, name=f"t1y_{img}_{j}",
                                tag="t1y", bufs=8)
                pav = pa[:].rearrange("p (a b) -> p a b", b=8)
                t1v = t1[:].rearrange("p (a b) -> p a b", b=8)
                # DC columns get the Y level shift; rest plain copy
                nc.vector.tensor_scalar_add(t1v[:, :, 0], pav[:, :, 0],
                                            -float(LS))
                nc.vector.tensor_copy(t1v[:, :, 1:8], pav[:, :, 1:8])
                S[img].setdefault("t1y", {})[j] = t1

            def st_AT1c(img, cn, jp):
                """fused A+T1 chroma: j-pair jp -> t1c[(cn, jp)]
                [128, 2, 256]."""
                x16 = S[img]["x16"]
                pa = psA.tile([128, W], f32, name=f"AT1c_{img}_{cn}_{jp}",
                              tag="psA")
                pav = pa[:].rearrange("p (a b) -> p a b", b=256)
                for jj in range(2):
                    j = 2 * jp + jj
                    for i in range(2):
                        for k in range(2):
                            for ci, wname in enumerate("rgb"):
                                mm(pav[:, jj,
                                       128 * i + 64 * k:128 * i + 64 * (k + 1)],
                                   x16[:, ci, 2 * i + k,
                                       128 * j:128 * (j + 1)],
                                   cs[f"a_{cn}_{wname}_k{k}"],
                                   ci == 0, ci == 2)
                t1 = wpool.tile([128, 2, 256], f32r,
                                name=f"t1c_{img}_{cn}_{jp}", tag="t1c",
                                bufs=8)
                nc.scalar.activation(t1[:], pav[:], ACT_COPY)
                S[img].setdefault("t1c", {})[(cn, jp)] = t1

            def st_BQy(img, i):
                t1 = S[img]["t1y"][i]
                pb = psB.tile([128, W], f32, name=f"B_{img}_y_{i}",
                              tag="psB")
                mm(pb[:], cs["b_y"], t1[:], True, True)
                pbv = pb[:].rearrange("p (a b) -> p a b", b=8)
                w1 = wpool.tile([128, W], f32, name=f"q1_{img}_y_{i}",
                                tag="q1", bufs=3)
                w1v = w1[:].rearrange("p (a b) -> p a b", b=8)
                nc.vector.tensor_tensor(w1v, pbv, bc8("ry", (64,)), MULT)
                w2 = wpool.tile([128, W], f32, name=f"q2_{img}_y_{i}",
                                tag="q2", bufs=3)
                nc.gpsimd.tensor_scalar(w2[:], w1[:], float(MAGIC),
                                        float(MAGIC), ADD, SUB)
                cq = wpool.tile([128, W], f16, name=f"cq_{img}_y_{i}",
                                tag="cqy", bufs=8)
                cqv = cq[:].rearrange("p (a b) -> p a b", b=8)
                w2v = w2[:].rearrange("p (a b) -> p a b", b=8)
                nc.vector.tensor_tensor(cqv, w2v, bc8("qy", (64,)), MULT)
                S[img].setdefault("cqy", {})[i] = cq

            def st_BQc(img, cn):
                t1c = S[img]["t1c"]
                pb0 = psB.tile([128, W], f32, name=f"B_{img}_{cn}",
                               tag="psB")
                pb = pb0[:].rearrange("p (c a) -> p c a", c=2)
                for b in range(2):
                    for k in range(2):
                        mm(pb[:, b, :], cs[f"b_c_k{k}"],
                           t1c[(cn, b)][:, k, :], k == 0, k == 1)
                pbv = pb0[:].rearrange("p (a b) -> p a b", b=8)
                w1 = wpool.tile([128, W], f32, name=f"q1_{img}_{cn}",
                                tag="q1c", bufs=2)
                w1v = w1[:].rearrange("p (a b) -> p a b", b=8)
                nc.vector.tensor_tensor(w1v, pbv, bc8("rc", (64,)), MULT)
                w2 = wpool.tile([128, W], f32, name=f"q2_{img}_{cn}",
                                tag="q2c", bufs=2)
                nc.gpsimd.tensor_scalar(w2[:], w1[:], float(MAGIC),
                                        float(MAGIC), ADD, SUB)
                cq = wpool.tile([128, 2, 256], f16, name=f"cq_{img}_{cn}",
                                tag="cqc", bufs=4)
                cqv = cq[:].rearrange("p c u -> p (c u)")\
                    .rearrange("p (a b) -> p a b", b=8)
                w2v = w2[:].rearrange("p (a b) -> p a b", b=8)
                nc.vector.tensor_tensor(cqv, w2v, bc8("qc", (64,)), MULT)
                S[img].setdefault("cqc", {})[cn] = cq

            def st_CT2y(img, j):
                cqy = S[img]["cqy"]
                pt = psT.tile([128, W], f32, name=f"CT2y_{img}_{j}",
                              tag="psT")
                for i in range(4):
                    mm(pt[:, 128 * i:128 * (i + 1)],
                       cqy[i][:, 128 * j:128 * (j + 1)], cs["bd"],
                       True, True)
                t2 = wpool.tile([128, W], f16, name=f"t2y_{img}_{j}",
                                tag="t2y", bufs=8)
                nc.scalar.activation(t2[:], pt[:], ACT_IDENT,
                                     bias=cs["bias_y"], scale=1.0)
                S[img].setdefault("t2y", {})[j] = t2

            def st_CT2c(img, cn, j):
                cq = S[img]["cqc"][cn]
                pt = psT.tile([128, W], f32, name=f"CT2c_{img}_{cn}_{j}",
                              tag="psT")
                for i in range(4):
                    mm(pt[:, 128 * i:128 * (i + 1)],
                       cq[:, i // 2, 128 * j:128 * (j + 1)],
                       cs[f"cc{i % 2}"], True, True)
                t2 = wpool.tile([128, W], f16, name=f"t2c_{img}_{cn}_{j}",
                                tag="t2c", bufs=8)
                nc.scalar.activation(t2[:], pt[:], ACT_COPY)
                S[img].setdefault("t2c", {})[(cn, j)] = t2

            def st_D(img, och_list=("r", "g", "b")):
                t2y = S[img]["t2y"]
                t2c = S[img]["t2c"]
                for och in och_list:
                    oi = "rgb".index(och)
                    ot = dpool.tile([128, 4, W], f32, name=f"ot_{img}_{och}",
                                    tag="ot")
                    for i in range(4):
                        pd = psD.tile([128, W], f32,
                                      name=f"D_{img}_{och}_{i}", tag="psD")
                        terms = list(W_BWD[och].items())
                        mm(pd[:], cs["dd_y"], t2y[i][:], True, False)
                        for ti, (cch, _) in enumerate(terms):
                            mm(pd[:], cs[f"d_{och}_{cch}_k{i % 2}"],
                               t2c[(cch, i // 2)][:],
                               False, ti == len(terms) - 1)
                        if i % 2 == 0:
                            fe = wpool.tile([128, W], f32,
                                            name=f"fin_{img}_{och}_{i}",
                                            tag="fin", bufs=2)
                            nc.scalar.activation(
                                fe[:], pd[:],
                                mybir.ActivationFunctionType.Relu)
                            nc.gpsimd.tensor_scalar_min(ot[:, i, :],
                                                        fe[:], 1.0)
                        else:
                            nc.vector.tensor_scalar(ot[:, i, :], pd[:],
                                                    1.0, 0.0, MIN, MAX)
                        nc.sync.dma_start(
                            out_d[img, oi, 128 * i:128 * (i + 1), :],
                            ot[:, i, :])

            # ---------------- emission schedule ----------------
            st_load(0)
            st_load(1)
            st_conv(0)
            for j in range(4):
                st_AT1y(0, j)
            for cn in ("cb", "cr"):
                for jp in range(2):
                    st_AT1c(0, cn, jp)
            st_conv(1)
            for i in range(4):
                st_BQy(0, i)
            st_BQc(0, "cb")
            st_BQc(0, "cr")
            for j in range(4):
                st_CT2y(0, j)
            for cn in ("cb", "cr"):
                for j in range(2):
                    st_CT2c(0, cn, j)
            st_D(0, ("r", "g"))
            for j in range(4):
                st_AT1y(1, j)
            st_D(0, ("b",))
            for cn in ("cb", "cr"):
                for jp in range(2):
                    st_AT1c(1, cn, jp)
            for i in range(4):
                st_BQy(1, i)
            st_BQc(1, "cb")
            st_BQc(1, "cr")
            for j in range(4):
                st_CT2y(1, j)
            for cn in ("cb", "cr"):
                for j in range(2):
                    st_CT2c(1, cn, j)
            st_D(1, ("r", "g", "b"))

    nc.compile()
    return nc


def kernel(x: np.ndarray) -> np.ndarray:
    global _PROGRAM, LAST_RESULT
    from concourse.bass_utils import run_bass_kernel_spmd

    x = np.ascontiguousarray(np.asarray(x, dtype=np.float32))
    assert x.shape == (N_CORES * IMGS_PER_CORE, 3, H, W)

    if _PROGRAM is None:
        _PROGRAM = _build_program()
    nc = _PROGRAM

    in_maps = []
    for c in range(N_CORES):
        m = {"xc": x[IMGS_PER_CORE * c:IMGS_PER_CORE * (c + 1)],
             "pack_r": _PACK_R, "pack_h": _PACK_H, "pack_f": _PACK_F}
        in_maps.append(m)

    res = run_bass_kernel_spmd(nc, in_maps, list(range(N_CORES)), trace=TRACE)
    LAST_RESULT = res
    out = np.concatenate([res.results[c]["outc"] for c in range(N_CORES)],
                         axis=0)
    return out
